# revision 36
# baseline (speedup 1.0000x reference)
import sys, os
sys.path.insert(0, '/opt/trn_rl_repo')
import numpy as np
import ml_dtypes

import concourse.bass as bass
from concourse import bacc
import concourse.mybir as mybir
from concourse.tile import TileContext
from concourse.bass_utils import run_bass_kernel_spmd

B, S = 2, 4096
HEADS, D = 8, 128
HID = HEADS * D
CHUNK = 64
NH = 4
NB = 256  # buckets per hash
N = NH * S          # 16384 sorted slots per (b,h)
NCH = N // CHUNK    # 256 chunks
EPS = 1e-6

f32 = mybir.dt.float32
bf16 = mybir.dt.bfloat16
u8 = mybir.dt.uint8


def _bf(x):
    return x.astype(ml_dtypes.bfloat16)


def _split_bf16(x):
    hi = x.astype(ml_dtypes.bfloat16).astype(np.float32)
    lo = x - hi
    return hi, lo


# ---------------- Launch 1: projections qk^T, v^T per (batch, 2 heads) ----------
def build_nc1():
    nc = bacc.Bacc()
    hT = nc.declare_dram_parameter("hT", [HID, S], f32, isOutput=False)
    wq_hi = nc.declare_dram_parameter("wq_hi", [HID, 2 * D], f32, isOutput=False)
    wq_lo = nc.declare_dram_parameter("wq_lo", [HID, 2 * D], f32, isOutput=False)
    wv = nc.declare_dram_parameter("wv", [HID, 2 * D], f32, isOutput=False)
    qk_out = nc.declare_dram_parameter("qk_out", [2 * D, S], f32, isOutput=True)
    v_out = nc.declare_dram_parameter("v_out", [2 * D, S], f32, isOutput=True)

    NBLK = 8
    BW = S // NBLK  # 512
    with TileContext(nc) as tc:
        with tc.tile_pool(name="w", bufs=1) as wp, \
             tc.tile_pool(name="h", bufs=3) as hp, \
             tc.tile_pool(name="st", bufs=3) as sp, \
             tc.tile_pool(name="ps", bufs=2, space="PSUM") as pp:
            # weights resident: bf16 for qk planes, f32r for v
            wqh_t = wp.tile([128, 8 * 2 * D], bf16, tag="wqh")
            wql_t = wp.tile([128, 8 * 2 * D], bf16, tag="wql")
            wv_t = wp.tile([128, 8 * 2 * D], mybir.dt.float32r, tag="wv")
            wqh = wqh_t[:].rearrange("p (k m) -> k p m", m=2 * D)
            wql = wql_t[:].rearrange("p (k m) -> k p m", m=2 * D)
            wvr = wv_t[:].rearrange("p (k m) -> k p m", m=2 * D)
            for k in range(8):
                ks = slice(k * 128, (k + 1) * 128)
                nc.gpsimd.dma_start(out=wqh[k], in_=wq_hi[ks, :])
                nc.gpsimd.dma_start(out=wql[k], in_=wq_lo[ks, :])
                nc.gpsimd.dma_start(out=wvr[k], in_=wv[ks, :])

            for blk in range(NBLK):
                ps_q = pp.tile([128, 2 * BW], f32, tag="psq")   # qk^T both heads
                ps_v = pp.tile([128, 2 * BW], f32, tag="psv")   # v^T both heads
                for k in range(8):
                    hf = hp.tile([128, BW], f32, tag="hf")
                    nc.sync.dma_start(out=hf[:], in_=hT[k * 128:(k + 1) * 128,
                                                       blk * BW:(blk + 1) * BW])
                    hhi = hp.tile([128, BW], bf16, tag="hhi")
                    nc.scalar.copy(hhi[:], hf[:])
                    hlo = hp.tile([128, BW], bf16, tag="hlo")
                    # hlo = hf - hhi  (one DVE pass)
                    nc.vector.scalar_tensor_tensor(
                        out=hlo[:], in0=hf[:], scalar=1.0, in1=hhi[:],
                        op0=mybir.AluOpType.mult, op1=mybir.AluOpType.subtract)
                    hr = hp.tile([128, BW], mybir.dt.float32r, tag="hr")
                    nc.scalar.copy(hr[:], hf[:])
                    st = (k == 0)
                    sp_ = (k == 7)
                    for h in range(2):
                        po = h * BW
                        wc = slice(h * D, (h + 1) * D)
                        nc.tensor.matmul(ps_q[:, po:po + BW], wqh[k, :, wc], hhi[:],
                                         start=st, stop=False)
                        nc.tensor.matmul(ps_q[:, po:po + BW], wqh[k, :, wc], hlo[:],
                                         start=False, stop=False)
                        nc.tensor.matmul(ps_q[:, po:po + BW], wql[k, :, wc], hhi[:],
                                         start=False, stop=sp_)
                        nc.tensor.matmul(ps_v[:, po:po + BW], wvr[k, :, wc], hr[:],
                                         start=st, stop=sp_)
                for h in range(2):
                    po = h * BW
                    oq = sp.tile([128, BW], f32, tag="oq")
                    ov = sp.tile([128, BW], f32, tag="ov")
                    nc.scalar.copy(oq[:], ps_q[:, po:po + BW])
                    nc.vector.tensor_copy(ov[:], ps_v[:, po:po + BW])
                    nc.sync.dma_start(out=qk_out[h * D:(h + 1) * D,
                                                 blk * BW:(blk + 1) * BW], in_=oq[:])
                    nc.sync.dma_start(out=v_out[h * D:(h + 1) * D,
                                                blk * BW:(blk + 1) * BW], in_=ov[:])
    nc.finalize()
    return nc


# ------------- Launch 2: chunked attention over pre-sorted data ------------------
# Per core inputs (per head h in {0,1}):
#  kx_{h}:  [128, 64+N] bf16   K^T extended (normalized keys, sorted, 64-wrap front)
#  qx_{h}:  [128, N]    bf16   Q^T sorted (unnormalized queries)
#  vx_{h}:  [64+N, 132] bf16   V extended rows (+col 128 = 1.0; 129..131 pad)
#  mk_{h}:  [128, N]    u8     mask (key-window x query) 1=attend
# Output: out_{h}: [N, 132] f32  (cols 0:128 out_unnorm, col 128 = sum)
def build_nc2():
    nc = bacc.Bacc()
    ins = {}
    for h in range(2):
        ins[f"kx_{h}"] = nc.declare_dram_parameter(f"kx_{h}", [128, 2 * N], bf16, isOutput=False)
        ins[f"qx_{h}"] = nc.declare_dram_parameter(f"qx_{h}", [128, N], bf16, isOutput=False)
        ins[f"vx_{h}"] = nc.declare_dram_parameter(f"vx_{h}", [257 * 64, 132], bf16, isOutput=False)
        ins[f"mk_{h}"] = nc.declare_dram_parameter(f"mk_{h}", [128, N], bf16, isOutput=False)
        ins[f"out_{h}"] = nc.declare_dram_parameter(f"out_{h}", [N, 132], f32, isOutput=True)

    BLK = 8                     # chunks per psum bank
    NBLK2 = NCH // BLK          # 32 blocks
    with TileContext(nc) as tc:
        for h in range(2):
            with tc.tile_pool(name=f"big{h}", bufs=1) as bigp, \
                 tc.tile_pool(name=f"wk{h}", bufs=3) as wkp, \
                 tc.tile_pool(name=f"ps{h}", bufs=2, space="PSUM") as psp, \
                 tc.tile_pool(name=f"ps2{h}", bufs=2, space="PSUM") as psp2:
                kx = bigp.tile([128, 2 * N], bf16, tag="kx")
                qx = bigp.tile([128, N], bf16, tag="qx")
                vx = bigp.tile([64, 257 * 132], bf16, tag="vx")
                nc.sync.dma_start(out=kx[:], in_=ins[f"kx_{h}"][:])
                nc.sync.dma_start(out=qx[:], in_=ins[f"qx_{h}"][:])
                vxv = vx[:].rearrange("p (w m) -> p w m", m=132)
                nc.sync.dma_start(
                    out=vxv, in_=ins[f"vx_{h}"][:].rearrange("(w p) m -> p w m", p=64))

                for blk in range(NBLK2):
                    dps = psp.tile([128, BLK * CHUNK], f32, tag="dps")
                    # MM1: per chunk j, keys = sorted slots [j*64, j*64+128) (ext),
                    # queries chunk j. lhsT = kx slice, rhs = qx chunk.
                    for jj in range(BLK):
                        j = blk * BLK + jj
                        nc.tensor.matmul(
                            dps[:, jj * CHUNK:(jj + 1) * CHUNK],
                            kx[:, j * 128:(j + 1) * 128],
                            qx[:, j * CHUNK:(j + 1) * CHUNK],
                            start=True, stop=True)
                    # exp
                    ex = wkp.tile([128, BLK * CHUNK], f32, tag="ex")
                    nc.scalar.activation(ex[:], dps[:],
                                         mybir.ActivationFunctionType.Exp)
                    # mask multiply -> A (bf16)
                    mks = wkp.tile([128, BLK * CHUNK], bf16, tag="mks")
                    nc.sync.dma_start(out=mks[:],
                                      in_=ins[f"mk_{h}"][:, blk * BLK * CHUNK:(blk + 1) * BLK * CHUNK])
                    at_lo = wkp.tile([64, BLK * CHUNK], bf16, tag="at_lo")
                    at_hi = wkp.tile([64, BLK * CHUNK], bf16, tag="at_hi")
                    nc.vector.tensor_tensor(out=at_lo[:], in0=ex[0:64, :], in1=mks[0:64, :],
                                            op=mybir.AluOpType.mult)
                    nc.vector.tensor_tensor(out=at_hi[:], in0=ex[64:128, :], in1=mks[64:128, :],
                                            op=mybir.AluOpType.mult)
                    # MM2 halves: out[q,d] += A_half.T @ V_half
                    for jj in range(0, BLK, 2):
                        j = blk * BLK + jj
                        ops = psp2.tile([128, 132], f32, tag="ops")
                        for q2 in range(2):  # two chunks -> two 64-row halves of psum
                            jq = j + q2
                            for half in range(2):  # key half w -> ext 64-row block jq+half
                                w = jq + half
                                at_sel = at_lo if (w % 2) == 0 else at_hi
                                a_h = at_sel[:, (jj + q2) * CHUNK:(jj + q2 + 1) * CHUNK]
                                v_h = vxv[:, w, :]
                                nc.tensor.matmul(ops[q2 * 64:q2 * 64 + 64, :],
                                                 a_h, v_h,
                                                 start=(half == 0), stop=(half == 1))
                        ob = wkp.tile([128, 132], f32, tag="ob")
                        nc.vector.tensor_copy(ob[:], ops[:])
                        nc.sync.dma_start(
                            out=ins[f"out_{h}"][j * CHUNK:(j + 2) * CHUNK, :].rearrange(
                                "(a p) m -> p (a m)", p=128),
                            in_=ob[:])
    nc.finalize()
    return nc


_NC1 = None
_NC2 = None
EXEC_NS = []   # per-launch exec_time_ns when BASS_TRACE is set
TRACE_DIRS = []


def _run(nc, in_maps, tag=""):
    import time
    t0 = time.time()
    r = run_bass_kernel_spmd(nc, in_maps, list(range(8)))
    EXEC_NS.append((tag, time.time() - t0))
    return r


def kernel(hidden_states, w_qk, w_v, rotations):
    global _NC1, _NC2
    hidden_states = np.asarray(hidden_states, dtype=np.float32)
    w_qk = np.asarray(w_qk, dtype=np.float32)
    w_v = np.asarray(w_v, dtype=np.float32)
    rotations = np.asarray(rotations, dtype=np.float32)

    # ---- launch 1: projections ----
    if _NC1 is None:
        _NC1 = build_nc1()
    in_maps1 = []
    for core in range(8):
        b = core // 4
        hp = core % 4  # head pair
        rows = slice(2 * hp * D, (2 * hp + 2) * D)
        wq = w_qk[rows, :].T.copy()          # [HID, 256]
        wqh, wql = _split_bf16(wq)
        in_maps1.append({
            "hT": np.ascontiguousarray(hidden_states[b].T),
            "wq_hi": wqh, "wq_lo": wql,
            "wv": np.ascontiguousarray(w_v[rows, :].T),
        })
    res1 = _run(_NC1, in_maps1, "nc1").results

    # ---- host middle: hashing, sort, layout prep (integer/index bookkeeping) ----
    rot2 = rotations.reshape(D, NH * (NB // 2))        # [128, 512]
    in_maps2 = []
    host_ctx = []
    for core in range(8):
        qk2 = res1[core]["qk_out"]          # [256, S] = qk^T two heads
        v2 = res1[core]["v_out"]
        m2 = {}
        ctx = []
        for h in range(2):
            qkT = qk2[h * D:(h + 1) * D, :]           # [128, S]
            qk = qkT.T                                 # [S, 128]
            v = v2[h * D:(h + 1) * D, :].T             # [S, 128]
            # hashing exactly like reference
            r = qk @ rot2                              # [S, 512]
            r = r.reshape(S, NH, NB // 2).transpose(1, 0, 2)   # [NH, S, 128]
            rc = np.concatenate([r, -r], axis=-1)      # [NH, S, 256]
            buckets = np.argmax(rc, axis=-1)           # [NH, S]
            buckets = buckets + (np.arange(NH) * NB)[:, None]
            flat = buckets.reshape(NH * S)
            sorted_idx = np.argsort(flat, kind="stable")       # [N]
            st = (sorted_idx % S).astype(np.int64)
            # normalized keys
            s_tok = (1.0 / np.sqrt(np.mean(qk * qk, axis=-1) + EPS)
                     / np.sqrt(np.float32(D))).astype(np.float32)
            k_norm = qk * s_tok[:, None]
            st_ext = np.concatenate([st[-CHUNK:], st])         # [64+N]
            kT = k_norm[st_ext].T                              # [128, 64+N]
            ext_rows = (np.arange(NCH)[:, None] * CHUNK + np.arange(128)[None, :])
            ccol = ext_rows % 128                              # dest col within chunk
            kxm = np.empty((128, NCH, 128), dtype=np.float32)
            np.put_along_axis(
                kxm.transpose(1, 2, 0), kT.T[ext_rows][:, :, :] * 0, ccol[:, :, None], axis=1) if False else None
            for j in range(NCH):
                kxm[:, j, ccol[j]] = kT[:, ext_rows[j]]
            kx = _bf(kxm.reshape(128, NCH * 128))
            qx = _bf(qk[st].T)                                 # [128, N]
            vs = v[st_ext]                                     # [64+N, 128]
            vx = np.zeros((257 * 64, 132), dtype=ml_dtypes.bfloat16)
            vx[:64 + N, :D] = _bf(vs)
            vx[:64 + N, D] = 1.0
            # mask [key 128, query N]: key k of chunk j is sorted slot j*64-64+k
            pos_ext = np.concatenate([st[-CHUNK:], st])
            qpos = st                                          # [N]
            kpos = np.empty((128, NCH), dtype=np.int64)
            for j in range(NCH):
                kpos[ccol[j], j] = pos_ext[ext_rows[j]]
            kpos = np.repeat(kpos, CHUNK, axis=1)              # [128, N]
            mask = (qpos[None, :] > kpos).astype(ml_dtypes.bfloat16)
            m2[f"kx_{h}"] = kx
            m2[f"qx_{h}"] = qx
            m2[f"vx_{h}"] = vx
            m2[f"mk_{h}"] = mask
            ctx.append((st, v))
        in_maps2.append(m2)
        host_ctx.append(ctx)

    if _NC2 is None:
        _NC2 = build_nc2()
    res2 = _run(_NC2, in_maps2, "nc2").results

    # ---- host: unsort, combine hash rounds ----
    out = np.zeros((B, S, HID), dtype=np.float32)
    for core in range(8):
        b = core // 4
        hp = core % 4
        for h in range(2):
            st, v = host_ctx[core][h]
            o = res2[core][f"out_{h}"]                 # [N, 132]
            ou = o[:, :D].reshape(NH, S, D)
            sm = o[:, D].reshape(NH, S)
            st4 = st.reshape(NH, S)
            # unsort each round
            ou_o = np.empty_like(ou)
            sm_o = np.empty_like(sm)
            for n in range(NH):
                ou_o[n, st4[n]] = ou[n]
                sm_o[n, st4[n]] = sm[n]
            lg = np.log(np.maximum(sm_o, 1e-38))
            lse = np.logaddexp.reduce(lg, axis=0)
            w = np.exp(lg - lse) / np.maximum(sm_o, 1e-38)     # [NH, S]
            res = np.sum(ou_o * w[:, :, None], axis=0)         # [S, D]
            dead = np.all(sm_o <= 1e-37, axis=0)
            if dead.any():
                res[dead] = v[dead]
            out[b, :, (2 * hp + h) * D:(2 * hp + h + 1) * D] = res
    return out



# revision 38
# speedup vs baseline: 1.3913x; 1.3913x over previous
import sys, os
sys.path.insert(0, '/opt/trn_rl_repo')
import numpy as np
import ml_dtypes

import concourse.bass as bass
from concourse import bacc
import concourse.mybir as mybir
from concourse.tile import TileContext
from concourse.bass_utils import run_bass_kernel_spmd

B, S = 2, 4096
HEADS, D = 8, 128
HID = HEADS * D
CHUNK = 64
NH = 4
NB = 256  # buckets per hash
N = NH * S          # 16384 sorted slots per (b,h)
NCH = N // CHUNK    # 256 chunks
EPS = 1e-6

f32 = mybir.dt.float32
bf16 = mybir.dt.bfloat16
u8 = mybir.dt.uint8


def _bf(x):
    return x.astype(ml_dtypes.bfloat16)


def _split_bf16(x):
    hi = x.astype(ml_dtypes.bfloat16).astype(np.float32)
    lo = x - hi
    return hi, lo


# ---------------- Launch 1: projections qk^T, v^T per (batch, 2 heads) ----------
def build_nc1():
    nc = bacc.Bacc()
    hT = nc.declare_dram_parameter("hT", [HID, S], f32, isOutput=False)
    wq_hi = nc.declare_dram_parameter("wq_hi", [HID, 2 * D], f32, isOutput=False)
    wq_lo = nc.declare_dram_parameter("wq_lo", [HID, 2 * D], f32, isOutput=False)
    wv = nc.declare_dram_parameter("wv", [HID, 2 * D], f32, isOutput=False)
    qk_out = nc.declare_dram_parameter("qk_out", [2 * D, S], f32, isOutput=True)
    v_out = nc.declare_dram_parameter("v_out", [2 * D, S], f32, isOutput=True)

    NBLK = 8
    BW = S // NBLK  # 512
    with TileContext(nc) as tc:
        with tc.tile_pool(name="w", bufs=1) as wp, \
             tc.tile_pool(name="h", bufs=3) as hp, \
             tc.tile_pool(name="st", bufs=3) as sp, \
             tc.tile_pool(name="ps", bufs=2, space="PSUM") as pp:
            # weights resident: bf16 for qk planes, f32r for v
            wqh_t = wp.tile([128, 8 * 2 * D], bf16, tag="wqh")
            wql_t = wp.tile([128, 8 * 2 * D], bf16, tag="wql")
            wv_t = wp.tile([128, 8 * 2 * D], mybir.dt.float32r, tag="wv")
            wqh = wqh_t[:].rearrange("p (k m) -> k p m", m=2 * D)
            wql = wql_t[:].rearrange("p (k m) -> k p m", m=2 * D)
            wvr = wv_t[:].rearrange("p (k m) -> k p m", m=2 * D)
            for k in range(8):
                ks = slice(k * 128, (k + 1) * 128)
                nc.gpsimd.dma_start(out=wqh[k], in_=wq_hi[ks, :])
                nc.gpsimd.dma_start(out=wql[k], in_=wq_lo[ks, :])
                nc.gpsimd.dma_start(out=wvr[k], in_=wv[ks, :])

            for blk in range(NBLK):
                ps_q = pp.tile([128, 2 * BW], f32, tag="psq")   # qk^T both heads
                ps_v = pp.tile([128, 2 * BW], f32, tag="psv")   # v^T both heads
                for k in range(8):
                    hf = hp.tile([128, BW], f32, tag="hf")
                    nc.sync.dma_start(out=hf[:], in_=hT[k * 128:(k + 1) * 128,
                                                       blk * BW:(blk + 1) * BW])
                    hhi = hp.tile([128, BW], bf16, tag="hhi")
                    nc.scalar.copy(hhi[:], hf[:])
                    hlo = hp.tile([128, BW], bf16, tag="hlo")
                    # hlo = hf - hhi  (one DVE pass)
                    nc.vector.scalar_tensor_tensor(
                        out=hlo[:], in0=hf[:], scalar=1.0, in1=hhi[:],
                        op0=mybir.AluOpType.mult, op1=mybir.AluOpType.subtract)
                    hr = hp.tile([128, BW], mybir.dt.float32r, tag="hr")
                    nc.scalar.copy(hr[:], hf[:])
                    st = (k == 0)
                    sp_ = (k == 7)
                    for h in range(2):
                        po = h * BW
                        wc = slice(h * D, (h + 1) * D)
                        nc.tensor.matmul(ps_q[:, po:po + BW], wqh[k, :, wc], hhi[:],
                                         start=st, stop=False)
                        nc.tensor.matmul(ps_q[:, po:po + BW], wqh[k, :, wc], hlo[:],
                                         start=False, stop=False)
                        nc.tensor.matmul(ps_q[:, po:po + BW], wql[k, :, wc], hhi[:],
                                         start=False, stop=sp_)
                        nc.tensor.matmul(ps_v[:, po:po + BW], wvr[k, :, wc], hr[:],
                                         start=st, stop=sp_)
                for h in range(2):
                    po = h * BW
                    oq = sp.tile([128, BW], f32, tag="oq")
                    ov = sp.tile([128, BW], f32, tag="ov")
                    nc.scalar.copy(oq[:], ps_q[:, po:po + BW])
                    nc.vector.tensor_copy(ov[:], ps_v[:, po:po + BW])
                    nc.sync.dma_start(out=qk_out[h * D:(h + 1) * D,
                                                 blk * BW:(blk + 1) * BW], in_=oq[:])
                    nc.sync.dma_start(out=v_out[h * D:(h + 1) * D,
                                                blk * BW:(blk + 1) * BW], in_=ov[:])
    nc.finalize()
    return nc


# ------------- Launch 2: chunked attention over pre-sorted data ------------------
# Per core inputs (per head h in {0,1}):
#  kx_{h}:  [128, 64+N] bf16   K^T extended (normalized keys, sorted, 64-wrap front)
#  qx_{h}:  [128, N]    bf16   Q^T sorted (unnormalized queries)
#  vx_{h}:  [64+N, 132] bf16   V extended rows (+col 128 = 1.0; 129..131 pad)
#  mk_{h}:  [128, N]    u8     mask (key-window x query) 1=attend
# Output: out_{h}: [N, 132] f32  (cols 0:128 out_unnorm, col 128 = sum)
def build_nc2():
    nc = bacc.Bacc()
    ins = {}
    for h in range(2):
        ins[f"kx_{h}"] = nc.declare_dram_parameter(f"kx_{h}", [128, 2 * N], bf16, isOutput=False)
        ins[f"qx_{h}"] = nc.declare_dram_parameter(f"qx_{h}", [128, N], bf16, isOutput=False)
        ins[f"vx_{h}"] = nc.declare_dram_parameter(f"vx_{h}", [257 * 64, 132], bf16, isOutput=False)
        ins[f"mk_{h}"] = nc.declare_dram_parameter(f"mk_{h}", [128, N], bf16, isOutput=False)
        ins[f"out_{h}"] = nc.declare_dram_parameter(f"out_{h}", [N, 132], f32, isOutput=True)

    BLK = 8                     # chunks per psum bank
    NBLK2 = NCH // BLK          # 32 blocks
    with TileContext(nc) as tc:
        for h in range(2):
            with tc.tile_pool(name=f"big{h}", bufs=1) as bigp, \
                 tc.tile_pool(name=f"wk{h}", bufs=3) as wkp, \
                 tc.tile_pool(name=f"ps{h}", bufs=2, space="PSUM") as psp, \
                 tc.tile_pool(name=f"ps2{h}", bufs=2, space="PSUM") as psp2:
                kx = bigp.tile([128, 2 * N], bf16, tag="kx")
                qx = bigp.tile([128, N], bf16, tag="qx")
                vx = bigp.tile([64, 257 * 132], bf16, tag="vx")
                nc.sync.dma_start(out=kx[:], in_=ins[f"kx_{h}"][:])
                nc.sync.dma_start(out=qx[:], in_=ins[f"qx_{h}"][:])
                vxv = vx[:].rearrange("p (w m) -> p w m", m=132)
                nc.sync.dma_start(
                    out=vxv, in_=ins[f"vx_{h}"][:].rearrange("(w p) m -> p w m", p=64))

                for blk in range(NBLK2):
                    dps = psp.tile([128, BLK * CHUNK], f32, tag="dps")
                    # MM1: per chunk j, keys = sorted slots [j*64, j*64+128) (ext),
                    # queries chunk j. lhsT = kx slice, rhs = qx chunk.
                    for jj in range(BLK):
                        j = blk * BLK + jj
                        nc.tensor.matmul(
                            dps[:, jj * CHUNK:(jj + 1) * CHUNK],
                            kx[:, j * 128:(j + 1) * 128],
                            qx[:, j * CHUNK:(j + 1) * CHUNK],
                            start=True, stop=True)
                    # exp
                    ex = wkp.tile([128, BLK * CHUNK], f32, tag="ex")
                    nc.scalar.activation(ex[:], dps[:],
                                         mybir.ActivationFunctionType.Exp)
                    # mask multiply -> A (bf16)
                    mks = wkp.tile([128, BLK * CHUNK], bf16, tag="mks")
                    nc.sync.dma_start(out=mks[:],
                                      in_=ins[f"mk_{h}"][:, blk * BLK * CHUNK:(blk + 1) * BLK * CHUNK])
                    at_lo = wkp.tile([64, BLK * CHUNK], bf16, tag="at_lo")
                    at_hi = wkp.tile([64, BLK * CHUNK], bf16, tag="at_hi")
                    nc.vector.tensor_tensor(out=at_lo[:], in0=ex[0:64, :], in1=mks[0:64, :],
                                            op=mybir.AluOpType.mult)
                    nc.vector.tensor_tensor(out=at_hi[:], in0=ex[64:128, :], in1=mks[64:128, :],
                                            op=mybir.AluOpType.mult)
                    # MM2 halves: out[q,d] += A_half.T @ V_half
                    for jj in range(0, BLK, 2):
                        j = blk * BLK + jj
                        ops = psp2.tile([128, 132], f32, tag="ops")
                        for q2 in range(2):  # two chunks -> two 64-row halves of psum
                            jq = j + q2
                            for half in range(2):  # key half w -> ext 64-row block jq+half
                                w = jq + half
                                at_sel = at_lo if (w % 2) == 0 else at_hi
                                a_h = at_sel[:, (jj + q2) * CHUNK:(jj + q2 + 1) * CHUNK]
                                v_h = vxv[:, w, :]
                                nc.tensor.matmul(ops[q2 * 64:q2 * 64 + 64, :],
                                                 a_h, v_h,
                                                 start=(half == 0), stop=(half == 1))
                        ob = wkp.tile([128, 132], f32, tag="ob")
                        nc.vector.tensor_copy(ob[:], ops[:])
                        nc.sync.dma_start(
                            out=ins[f"out_{h}"][j * CHUNK:(j + 2) * CHUNK, :].rearrange(
                                "(a p) m -> p (a m)", p=128),
                            in_=ob[:])
    nc.finalize()
    return nc


_NC1 = None
_NC2 = None
EXEC_NS = []   # per-launch exec_time_ns when BASS_TRACE is set
TRACE_DIRS = []

# SRC_IDX[j, p] = ext slot index of the key row that chunk j places at
# within-chunk column p (the (j*64)%128 rotation of the baseline layout)
_J = np.arange(NCH)[:, None]
_P = np.arange(128)[None, :]
SRC_IDX = _J * CHUNK + ((_P - _J * CHUNK) % 128)


def _run(nc, in_maps, tag=""):
    import time
    t0 = time.time()
    r = run_bass_kernel_spmd(nc, in_maps, list(range(8)))
    EXEC_NS.append((tag, time.time() - t0))
    return r


def kernel(hidden_states, w_qk, w_v, rotations):
    global _NC1, _NC2
    hidden_states = np.asarray(hidden_states, dtype=np.float32)
    w_qk = np.asarray(w_qk, dtype=np.float32)
    w_v = np.asarray(w_v, dtype=np.float32)
    rotations = np.asarray(rotations, dtype=np.float32)

    # ---- launch 1: projections ----
    if _NC1 is None:
        _NC1 = build_nc1()
    in_maps1 = []
    for core in range(8):
        b = core // 4
        hp = core % 4  # head pair
        rows = slice(2 * hp * D, (2 * hp + 2) * D)
        wq = w_qk[rows, :].T.copy()          # [HID, 256]
        wqh, wql = _split_bf16(wq)
        in_maps1.append({
            "hT": np.ascontiguousarray(hidden_states[b].T),
            "wq_hi": wqh, "wq_lo": wql,
            "wv": np.ascontiguousarray(w_v[rows, :].T),
        })
    res1 = _run(_NC1, in_maps1, "nc1").results

    # ---- host middle: hashing, sort, layout prep (integer/index bookkeeping) ----
    rot2 = rotations.reshape(D, NH * (NB // 2))        # [128, 512]
    in_maps2 = []
    host_ctx = []
    for core in range(8):
        qk2 = res1[core]["qk_out"]          # [256, S] = qk^T two heads
        v2 = res1[core]["v_out"]
        m2 = {}
        ctx = []
        for h in range(2):
            qkT = qk2[h * D:(h + 1) * D, :]           # [128, S]
            qk = qkT.T                                 # [S, 128]
            v = v2[h * D:(h + 1) * D, :].T             # [S, 128]
            # hashing exactly like reference
            r = qk @ rot2                              # [S, 512]
            r = r.reshape(S, NH, NB // 2).transpose(1, 0, 2)   # [NH, S, 128]
            rc = np.concatenate([r, -r], axis=-1)      # [NH, S, 256]
            buckets = np.argmax(rc, axis=-1)           # [NH, S]
            buckets = buckets + (np.arange(NH) * NB)[:, None]
            flat = buckets.reshape(NH * S)
            sorted_idx = np.argsort(flat, kind="stable")       # [N]
            st = (sorted_idx % S).astype(np.int64)
            # normalized keys
            s_tok = (1.0 / np.sqrt(np.mean(qk * qk, axis=-1) + EPS)
                     / np.sqrt(np.float32(D))).astype(np.float32)
            k_norm = qk * s_tok[:, None]
            st_ext = np.concatenate([st[-CHUNK:], st])         # [64+N]
            # chunk j's window occupies kx cols [j*128, (j+1)*128); the row
            # landing at within-chunk col p is ext slot j*64 + ((p-j*64)%128).
            # SRC[j, p] is that ext slot -- one fancy-index builds kx/kpos.
            kbf = _bf(k_norm)[st_ext]                          # [64+N, 128] bf16
            kxm = kbf[SRC_IDX]                                 # [NCH, 128, 128]
            kx = np.ascontiguousarray(
                kxm.transpose(2, 0, 1).reshape(128, NCH * 128))
            qx = np.ascontiguousarray(_bf(qk)[st].T)           # [128, N]
            vs = v[st_ext]                                     # [64+N, 128]
            vx = np.zeros((257 * 64, 132), dtype=ml_dtypes.bfloat16)
            vx[:64 + N, :D] = _bf(vs)
            vx[:64 + N, D] = 1.0
            # mask [key 128, query N]: qpos > kpos on original positions
            qpos = st                                          # [N]
            kpos = np.repeat(st_ext[SRC_IDX].transpose(1, 0), CHUNK, axis=1)
            mask = (qpos[None, :] > kpos).astype(ml_dtypes.bfloat16)
            m2[f"kx_{h}"] = kx
            m2[f"qx_{h}"] = qx
            m2[f"vx_{h}"] = vx
            m2[f"mk_{h}"] = mask
            ctx.append((st, v))
        in_maps2.append(m2)
        host_ctx.append(ctx)

    if _NC2 is None:
        _NC2 = build_nc2()
    res2 = _run(_NC2, in_maps2, "nc2").results

    # ---- host: unsort, combine hash rounds ----
    out = np.zeros((B, S, HID), dtype=np.float32)
    for core in range(8):
        b = core // 4
        hp = core % 4
        for h in range(2):
            st, v = host_ctx[core][h]
            o = res2[core][f"out_{h}"]                 # [N, 132]
            ou = o[:, :D].reshape(NH, S, D)
            sm = o[:, D].reshape(NH, S)
            st4 = st.reshape(NH, S)
            # unsort each round
            ou_o = np.empty_like(ou)
            sm_o = np.empty_like(sm)
            for n in range(NH):
                ou_o[n, st4[n]] = ou[n]
                sm_o[n, st4[n]] = sm[n]
            lg = np.log(np.maximum(sm_o, 1e-38))
            lse = np.logaddexp.reduce(lg, axis=0)
            w = np.exp(lg - lse) / np.maximum(sm_o, 1e-38)     # [NH, S]
            res = np.sum(ou_o * w[:, :, None], axis=0)         # [S, D]
            dead = np.all(sm_o <= 1e-37, axis=0)
            if dead.any():
                res[dead] = v[dead]
            out[b, :, (2 * hp + h) * D:(2 * hp + h + 1) * D] = res
    return out



# revision 41
# speedup vs baseline: 1.6426x; 1.1806x over previous
import sys, os
sys.path.insert(0, '/opt/trn_rl_repo')
import numpy as np
import ml_dtypes

import concourse.bass as bass
from concourse import bacc
import concourse.mybir as mybir
from concourse.tile import TileContext
from concourse.bass_utils import run_bass_kernel_spmd

B, S = 2, 4096
HEADS, D = 8, 128
HID = HEADS * D
CHUNK = 64
NH = 4
NB = 256  # buckets per hash
N = NH * S          # 16384 sorted slots per (b,h)
NCH = N // CHUNK    # 256 chunks
EPS = 1e-6

f32 = mybir.dt.float32
bf16 = mybir.dt.bfloat16
u8 = mybir.dt.uint8


def _bf(x):
    return x.astype(ml_dtypes.bfloat16)


def _split_bf16(x):
    hi = x.astype(ml_dtypes.bfloat16).astype(np.float32)
    lo = x - hi
    return hi, lo


# ---------------- Launch 1: projections qk^T, v^T per (batch, 2 heads) ----------
def build_nc1():
    nc = bacc.Bacc()
    hT = nc.declare_dram_parameter("hT", [HID, S], f32, isOutput=False)
    wq_hi = nc.declare_dram_parameter("wq_hi", [HID, 2 * D], f32, isOutput=False)
    wq_lo = nc.declare_dram_parameter("wq_lo", [HID, 2 * D], f32, isOutput=False)
    wv = nc.declare_dram_parameter("wv", [HID, 2 * D], f32, isOutput=False)
    qk_out = nc.declare_dram_parameter("qk_out", [2 * D, S], f32, isOutput=True)
    v_out = nc.declare_dram_parameter("v_out", [2 * D, S], f32, isOutput=True)

    NBLK = 8
    BW = S // NBLK  # 512
    with TileContext(nc) as tc:
        with tc.tile_pool(name="w", bufs=1) as wp, \
             tc.tile_pool(name="h", bufs=3) as hp, \
             tc.tile_pool(name="st", bufs=3) as sp, \
             tc.tile_pool(name="ps", bufs=2, space="PSUM") as pp:
            # weights resident: bf16 for qk planes, f32r for v
            wqh_t = wp.tile([128, 8 * 2 * D], bf16, tag="wqh")
            wql_t = wp.tile([128, 8 * 2 * D], bf16, tag="wql")
            wv_t = wp.tile([128, 8 * 2 * D], mybir.dt.float32r, tag="wv")
            wqh = wqh_t[:].rearrange("p (k m) -> k p m", m=2 * D)
            wql = wql_t[:].rearrange("p (k m) -> k p m", m=2 * D)
            wvr = wv_t[:].rearrange("p (k m) -> k p m", m=2 * D)
            for k in range(8):
                ks = slice(k * 128, (k + 1) * 128)
                nc.gpsimd.dma_start(out=wqh[k], in_=wq_hi[ks, :])
                nc.gpsimd.dma_start(out=wql[k], in_=wq_lo[ks, :])
                nc.gpsimd.dma_start(out=wvr[k], in_=wv[ks, :])

            for blk in range(NBLK):
                ps_q = pp.tile([128, 2 * BW], f32, tag="psq")   # qk^T both heads
                ps_v = pp.tile([128, 2 * BW], f32, tag="psv")   # v^T both heads
                for k in range(8):
                    hf = hp.tile([128, BW], f32, tag="hf")
                    nc.sync.dma_start(out=hf[:], in_=hT[k * 128:(k + 1) * 128,
                                                       blk * BW:(blk + 1) * BW])
                    hhi = hp.tile([128, BW], bf16, tag="hhi")
                    nc.scalar.copy(hhi[:], hf[:])
                    hlo = hp.tile([128, BW], bf16, tag="hlo")
                    # hlo = hf - hhi  (one DVE pass)
                    nc.vector.scalar_tensor_tensor(
                        out=hlo[:], in0=hf[:], scalar=1.0, in1=hhi[:],
                        op0=mybir.AluOpType.mult, op1=mybir.AluOpType.subtract)
                    hr = hp.tile([128, BW], mybir.dt.float32r, tag="hr")
                    nc.scalar.copy(hr[:], hf[:])
                    st = (k == 0)
                    sp_ = (k == 7)
                    for h in range(2):
                        po = h * BW
                        wc = slice(h * D, (h + 1) * D)
                        nc.tensor.matmul(ps_q[:, po:po + BW], wqh[k, :, wc], hhi[:],
                                         start=st, stop=False)
                        nc.tensor.matmul(ps_q[:, po:po + BW], wqh[k, :, wc], hlo[:],
                                         start=False, stop=False)
                        nc.tensor.matmul(ps_q[:, po:po + BW], wql[k, :, wc], hhi[:],
                                         start=False, stop=sp_)
                        nc.tensor.matmul(ps_v[:, po:po + BW], wvr[k, :, wc], hr[:],
                                         start=st, stop=sp_)
                for h in range(2):
                    po = h * BW
                    oq = sp.tile([128, BW], f32, tag="oq")
                    ov = sp.tile([128, BW], f32, tag="ov")
                    nc.scalar.copy(oq[:], ps_q[:, po:po + BW])
                    nc.vector.tensor_copy(ov[:], ps_v[:, po:po + BW])
                    nc.sync.dma_start(out=qk_out[h * D:(h + 1) * D,
                                                 blk * BW:(blk + 1) * BW], in_=oq[:])
                    nc.sync.dma_start(out=v_out[h * D:(h + 1) * D,
                                                blk * BW:(blk + 1) * BW], in_=ov[:])
    nc.finalize()
    return nc


# ------------- Launch 2: chunked attention over pre-sorted data ------------------
# Per core inputs (per head h in {0,1}):
#  kx_{h}:  [128, 64+N] bf16   K^T extended (normalized keys, sorted, 64-wrap front)
#  qx_{h}:  [128, N]    bf16   Q^T sorted (unnormalized queries)
#  vx_{h}:  [64+N, 132] bf16   V extended rows (+col 128 = 1.0; 129..131 pad)
#  mk_{h}:  [128, N]    u8     mask (key-window x query) 1=attend
# Output: out_{h}: [N, 132] f32  (cols 0:128 out_unnorm, col 128 = sum)
def build_nc2():
    nc = bacc.Bacc()
    ins = {}
    for h in range(2):
        ins[f"kx_{h}"] = nc.declare_dram_parameter(f"kx_{h}", [128, 2 * N], bf16, isOutput=False)
        ins[f"qx_{h}"] = nc.declare_dram_parameter(f"qx_{h}", [128, N], bf16, isOutput=False)
        ins[f"vx_{h}"] = nc.declare_dram_parameter(f"vx_{h}", [257 * 64, 132], bf16, isOutput=False)
        ins[f"mk_{h}"] = nc.declare_dram_parameter(f"mk_{h}", [128, N], bf16, isOutput=False)
        ins[f"out_{h}"] = nc.declare_dram_parameter(f"out_{h}", [N, 132], bf16, isOutput=True)

    BLK = 8                     # chunks per psum bank
    NBLK2 = NCH // BLK          # 32 blocks
    with TileContext(nc) as tc:
        for h in range(2):
            with tc.tile_pool(name=f"big{h}", bufs=1) as bigp, \
                 tc.tile_pool(name=f"wk{h}", bufs=3) as wkp, \
                 tc.tile_pool(name=f"ps{h}", bufs=2, space="PSUM") as psp, \
                 tc.tile_pool(name=f"ps2{h}", bufs=2, space="PSUM") as psp2:
                kx = bigp.tile([128, 2 * N], bf16, tag="kx")
                qx = bigp.tile([128, N], bf16, tag="qx")
                vx = bigp.tile([64, 257 * 132], bf16, tag="vx")
                nc.sync.dma_start(out=kx[:], in_=ins[f"kx_{h}"][:])
                nc.sync.dma_start(out=qx[:], in_=ins[f"qx_{h}"][:])
                vxv = vx[:].rearrange("p (w m) -> p w m", m=132)
                nc.sync.dma_start(
                    out=vxv, in_=ins[f"vx_{h}"][:].rearrange("(w p) m -> p w m", p=64))

                for blk in range(NBLK2):
                    dps = psp.tile([128, BLK * CHUNK], f32, tag="dps")
                    # MM1: per chunk j, keys = sorted slots [j*64, j*64+128) (ext),
                    # queries chunk j. lhsT = kx slice, rhs = qx chunk.
                    for jj in range(BLK):
                        j = blk * BLK + jj
                        nc.tensor.matmul(
                            dps[:, jj * CHUNK:(jj + 1) * CHUNK],
                            kx[:, j * 128:(j + 1) * 128],
                            qx[:, j * CHUNK:(j + 1) * CHUNK],
                            start=True, stop=True)
                    # exp
                    ex = wkp.tile([128, BLK * CHUNK], f32, tag="ex")
                    nc.scalar.activation(ex[:], dps[:],
                                         mybir.ActivationFunctionType.Exp)
                    # mask multiply -> A (bf16)
                    mks = wkp.tile([128, BLK * CHUNK], bf16, tag="mks")
                    nc.sync.dma_start(out=mks[:],
                                      in_=ins[f"mk_{h}"][:, blk * BLK * CHUNK:(blk + 1) * BLK * CHUNK])
                    at_lo = wkp.tile([64, BLK * CHUNK], bf16, tag="at_lo")
                    at_hi = wkp.tile([64, BLK * CHUNK], bf16, tag="at_hi")
                    nc.vector.tensor_tensor(out=at_lo[:], in0=ex[0:64, :], in1=mks[0:64, :],
                                            op=mybir.AluOpType.mult)
                    nc.vector.tensor_tensor(out=at_hi[:], in0=ex[64:128, :], in1=mks[64:128, :],
                                            op=mybir.AluOpType.mult)
                    # MM2 halves: out[q,d] += A_half.T @ V_half
                    for jj in range(0, BLK, 2):
                        j = blk * BLK + jj
                        ops = psp2.tile([128, 132], f32, tag="ops")
                        for q2 in range(2):  # two chunks -> two 64-row halves of psum
                            jq = j + q2
                            for half in range(2):  # key half w -> ext 64-row block jq+half
                                w = jq + half
                                at_sel = at_lo if (w % 2) == 0 else at_hi
                                a_h = at_sel[:, (jj + q2) * CHUNK:(jj + q2 + 1) * CHUNK]
                                v_h = vxv[:, w, :]
                                nc.tensor.matmul(ops[q2 * 64:q2 * 64 + 64, :],
                                                 a_h, v_h,
                                                 start=(half == 0), stop=(half == 1))
                        ob = wkp.tile([128, 132], bf16, tag="ob")
                        nc.vector.tensor_copy(ob[:], ops[:])
                        nc.sync.dma_start(
                            out=ins[f"out_{h}"][j * CHUNK:(j + 2) * CHUNK, :].rearrange(
                                "(a p) m -> p (a m)", p=128),
                            in_=ob[:])
    nc.finalize()
    return nc


_NC1 = None
_NC2 = None
EXEC_NS = []   # per-launch exec_time_ns when BASS_TRACE is set
TRACE_DIRS = []

# SRC_IDX[j, p] = ext slot index of the key row that chunk j places at
# within-chunk column p (the (j*64)%128 rotation of the baseline layout)
_J = np.arange(NCH)[:, None]
_P = np.arange(128)[None, :]
SRC_IDX = _J * CHUNK + ((_P - _J * CHUNK) % 128)


def _run(nc, in_maps, tag=""):
    import time
    t0 = time.time()
    r = run_bass_kernel_spmd(nc, in_maps, list(range(8)))
    EXEC_NS.append((tag, time.time() - t0))
    return r


def kernel(hidden_states, w_qk, w_v, rotations):
    global _NC1, _NC2
    hidden_states = np.asarray(hidden_states, dtype=np.float32)
    w_qk = np.asarray(w_qk, dtype=np.float32)
    w_v = np.asarray(w_v, dtype=np.float32)
    rotations = np.asarray(rotations, dtype=np.float32)

    # ---- launch 1: projections ----
    if _NC1 is None:
        _NC1 = build_nc1()
    in_maps1 = []
    for core in range(8):
        b = core // 4
        hp = core % 4  # head pair
        rows = slice(2 * hp * D, (2 * hp + 2) * D)
        wq = w_qk[rows, :].T.copy()          # [HID, 256]
        wqh, wql = _split_bf16(wq)
        in_maps1.append({
            "hT": np.ascontiguousarray(hidden_states[b].T),
            "wq_hi": wqh, "wq_lo": wql,
            "wv": np.ascontiguousarray(w_v[rows, :].T),
        })
    res1 = _run(_NC1, in_maps1, "nc1").results

    # ---- host middle: hashing, sort, layout prep (integer/index bookkeeping) ----
    rot2 = rotations.reshape(D, NH * (NB // 2))        # [128, 512]
    in_maps2 = []
    host_ctx = []
    for core in range(8):
        qk2 = res1[core]["qk_out"]          # [256, S] = qk^T two heads
        v2 = res1[core]["v_out"]
        m2 = {}
        ctx = []
        for h in range(2):
            qkT = qk2[h * D:(h + 1) * D, :]           # [128, S]
            qk = qkT.T                                 # [S, 128]
            v = v2[h * D:(h + 1) * D, :].T             # [S, 128]
            # hashing exactly like reference
            r = qk @ rot2                              # [S, 512]
            r = r.reshape(S, NH, NB // 2).transpose(1, 0, 2)   # [NH, S, 128]
            rc = np.concatenate([r, -r], axis=-1)      # [NH, S, 256]
            buckets = np.argmax(rc, axis=-1)           # [NH, S]
            buckets = buckets + (np.arange(NH) * NB)[:, None]
            flat = buckets.reshape(NH * S)
            sorted_idx = np.argsort(flat, kind="stable")       # [N]
            st = (sorted_idx % S).astype(np.int64)
            # normalized keys
            s_tok = (1.0 / np.sqrt(np.mean(qk * qk, axis=-1) + EPS)
                     / np.sqrt(np.float32(D))).astype(np.float32)
            k_norm = qk * s_tok[:, None]
            st_ext = np.concatenate([st[-CHUNK:], st])         # [64+N]
            # chunk j's window occupies kx cols [j*128, (j+1)*128); the row
            # landing at within-chunk col p is ext slot j*64 + ((p-j*64)%128).
            # SRC[j, p] is that ext slot -- one fancy-index builds kx/kpos.
            kbf = _bf(k_norm)[st_ext]                          # [64+N, 128] bf16
            kxm = kbf[SRC_IDX]                                 # [NCH, 128, 128]
            kx = np.ascontiguousarray(
                kxm.transpose(2, 0, 1).reshape(128, NCH * 128))
            qx = np.ascontiguousarray(_bf(qk)[st].T)           # [128, N]
            vs = v[st_ext]                                     # [64+N, 128]
            vx = np.zeros((257 * 64, 132), dtype=ml_dtypes.bfloat16)
            vx[:64 + N, :D] = _bf(vs)
            vx[:64 + N, D] = 1.0
            # mask [key 128, query N]: qpos > kpos on original positions
            qpos = st                                          # [N]
            kpos = np.repeat(st_ext[SRC_IDX].transpose(1, 0), CHUNK, axis=1)
            mask = (qpos[None, :] > kpos).astype(ml_dtypes.bfloat16)
            m2[f"kx_{h}"] = kx
            m2[f"qx_{h}"] = qx
            m2[f"vx_{h}"] = vx
            m2[f"mk_{h}"] = mask
            ctx.append((st, v))
        in_maps2.append(m2)
        host_ctx.append(ctx)

    if _NC2 is None:
        _NC2 = build_nc2()
    res2 = _run(_NC2, in_maps2, "nc2").results

    # ---- host: unsort, combine hash rounds ----
    out = np.zeros((B, S, HID), dtype=np.float32)
    for core in range(8):
        b = core // 4
        hp = core % 4
        for h in range(2):
            st, v = host_ctx[core][h]
            o = res2[core][f"out_{h}"].astype(np.float32)      # [N, 132]
            ou = o[:, :D].reshape(NH, S, D)
            sm = o[:, D].reshape(NH, S)
            st4 = st.reshape(NH, S)
            # unsort each round
            ou_o = np.empty_like(ou)
            sm_o = np.empty_like(sm)
            for n in range(NH):
                ou_o[n, st4[n]] = ou[n]
                sm_o[n, st4[n]] = sm[n]
            lg = np.log(np.maximum(sm_o, 1e-38))
            lse = np.logaddexp.reduce(lg, axis=0)
            w = np.exp(lg - lse) / np.maximum(sm_o, 1e-38)     # [NH, S]
            res = np.sum(ou_o * w[:, :, None], axis=0)         # [S, D]
            dead = np.all(sm_o <= 1e-37, axis=0)
            if dead.any():
                res[dead] = v[dead]
            out[b, :, (2 * hp + h) * D:(2 * hp + h + 1) * D] = res
    return out



# revision 43
# speedup vs baseline: 1.8575x; 1.1308x over previous
import sys, os
sys.path.insert(0, '/opt/trn_rl_repo')
import numpy as np
import ml_dtypes

import concourse.bass as bass
from concourse import bacc
import concourse.mybir as mybir
from concourse.tile import TileContext
from concourse.bass_utils import run_bass_kernel_spmd

B, S = 2, 4096
HEADS, D = 8, 128
HID = HEADS * D
CHUNK = 64
NH = 4
NB = 256  # buckets per hash
N = NH * S          # 16384 sorted slots per (b,h)
NCH = N // CHUNK    # 256 chunks
EPS = 1e-6

f32 = mybir.dt.float32
bf16 = mybir.dt.bfloat16
u8 = mybir.dt.uint8


def _bf(x):
    return x.astype(ml_dtypes.bfloat16)


def _split_bf16(x):
    hi = x.astype(ml_dtypes.bfloat16).astype(np.float32)
    lo = x - hi
    return hi, lo


# ---------------- Launch 1: projections qk^T, v^T per (batch, 2 heads) ----------
def build_nc1():
    nc = bacc.Bacc()
    hT = nc.declare_dram_parameter("hT", [HID, S], f32, isOutput=False)
    wq_hi = nc.declare_dram_parameter("wq_hi", [HID, 2 * D], f32, isOutput=False)
    wq_lo = nc.declare_dram_parameter("wq_lo", [HID, 2 * D], f32, isOutput=False)
    wv = nc.declare_dram_parameter("wv", [HID, 2 * D], f32, isOutput=False)
    qk_out = nc.declare_dram_parameter("qk_out", [2 * D, S], f32, isOutput=True)
    v_out = nc.declare_dram_parameter("v_out", [2 * D, S], f32, isOutput=True)

    NBLK = 8
    BW = S // NBLK  # 512
    with TileContext(nc) as tc:
        with tc.tile_pool(name="w", bufs=1) as wp, \
             tc.tile_pool(name="h", bufs=3) as hp, \
             tc.tile_pool(name="st", bufs=3) as sp, \
             tc.tile_pool(name="ps", bufs=2, space="PSUM") as pp:
            # weights resident: bf16 for qk planes, f32r for v
            wqh_t = wp.tile([128, 8 * 2 * D], bf16, tag="wqh")
            wql_t = wp.tile([128, 8 * 2 * D], bf16, tag="wql")
            wv_t = wp.tile([128, 8 * 2 * D], mybir.dt.float32r, tag="wv")
            wqh = wqh_t[:].rearrange("p (k m) -> k p m", m=2 * D)
            wql = wql_t[:].rearrange("p (k m) -> k p m", m=2 * D)
            wvr = wv_t[:].rearrange("p (k m) -> k p m", m=2 * D)
            for k in range(8):
                ks = slice(k * 128, (k + 1) * 128)
                nc.gpsimd.dma_start(out=wqh[k], in_=wq_hi[ks, :])
                nc.gpsimd.dma_start(out=wql[k], in_=wq_lo[ks, :])
                nc.gpsimd.dma_start(out=wvr[k], in_=wv[ks, :])

            for blk in range(NBLK):
                ps_q = pp.tile([128, 2 * BW], f32, tag="psq")   # qk^T both heads
                ps_v = pp.tile([128, 2 * BW], f32, tag="psv")   # v^T both heads
                for k in range(8):
                    hf = hp.tile([128, BW], f32, tag="hf")
                    nc.sync.dma_start(out=hf[:], in_=hT[k * 128:(k + 1) * 128,
                                                       blk * BW:(blk + 1) * BW])
                    hhi = hp.tile([128, BW], bf16, tag="hhi")
                    nc.scalar.copy(hhi[:], hf[:])
                    hlo = hp.tile([128, BW], bf16, tag="hlo")
                    # hlo = hf - hhi  (one DVE pass)
                    nc.vector.scalar_tensor_tensor(
                        out=hlo[:], in0=hf[:], scalar=1.0, in1=hhi[:],
                        op0=mybir.AluOpType.mult, op1=mybir.AluOpType.subtract)
                    hr = hp.tile([128, BW], mybir.dt.float32r, tag="hr")
                    nc.scalar.copy(hr[:], hf[:])
                    st = (k == 0)
                    sp_ = (k == 7)
                    for h in range(2):
                        po = h * BW
                        wc = slice(h * D, (h + 1) * D)
                        nc.tensor.matmul(ps_q[:, po:po + BW], wqh[k, :, wc], hhi[:],
                                         start=st, stop=False)
                        nc.tensor.matmul(ps_q[:, po:po + BW], wqh[k, :, wc], hlo[:],
                                         start=False, stop=False)
                        nc.tensor.matmul(ps_q[:, po:po + BW], wql[k, :, wc], hhi[:],
                                         start=False, stop=sp_)
                        nc.tensor.matmul(ps_v[:, po:po + BW], wvr[k, :, wc], hr[:],
                                         start=st, stop=sp_)
                for h in range(2):
                    po = h * BW
                    oq = sp.tile([128, BW], f32, tag="oq")
                    ov = sp.tile([128, BW], f32, tag="ov")
                    nc.scalar.copy(oq[:], ps_q[:, po:po + BW])
                    nc.vector.tensor_copy(ov[:], ps_v[:, po:po + BW])
                    nc.sync.dma_start(out=qk_out[h * D:(h + 1) * D,
                                                 blk * BW:(blk + 1) * BW], in_=oq[:])
                    nc.sync.dma_start(out=v_out[h * D:(h + 1) * D,
                                                blk * BW:(blk + 1) * BW], in_=ov[:])
    nc.finalize()
    return nc


# ------------- Launch 2: chunked attention over pre-sorted data ------------------
# Per core inputs (per head h in {0,1}):
#  kx_{h}:  [128, 64+N] bf16   K^T extended (normalized keys, sorted, 64-wrap front)
#  qx_{h}:  [128, N]    bf16   Q^T sorted (unnormalized queries)
#  vx_{h}:  [64+N, 132] bf16   V extended rows (+col 128 = 1.0; 129..131 pad)
#  mk_{h}:  [128, N]    u8     mask (key-window x query) 1=attend
# Output: out_{h}: [N, 132] f32  (cols 0:128 out_unnorm, col 128 = sum)
def build_nc2():
    nc = bacc.Bacc()
    ins = {}
    for h in range(2):
        ins[f"kx_{h}"] = nc.declare_dram_parameter(f"kx_{h}", [128, 2 * N], bf16, isOutput=False)
        ins[f"qx_{h}"] = nc.declare_dram_parameter(f"qx_{h}", [128, N], bf16, isOutput=False)
        ins[f"vx_{h}"] = nc.declare_dram_parameter(f"vx_{h}", [257 * 64, 132], bf16, isOutput=False)
        ins[f"kp_{h}"] = nc.declare_dram_parameter(f"kp_{h}", [128, NCH], mybir.dt.float32, isOutput=False)
        ins[f"stf_{h}"] = nc.declare_dram_parameter(f"stf_{h}", [1, N], mybir.dt.float32, isOutput=False)
        ins[f"out_{h}"] = nc.declare_dram_parameter(f"out_{h}", [N, 132], bf16, isOutput=True)

    BLK = 8                     # chunks per psum bank
    NBLK2 = NCH // BLK          # 32 blocks
    with TileContext(nc) as tc:
        for h in range(2):
            with tc.tile_pool(name=f"big{h}", bufs=1) as bigp, \
                 tc.tile_pool(name=f"wk{h}", bufs=3) as wkp, \
                 tc.tile_pool(name=f"ps{h}", bufs=2, space="PSUM") as psp, \
                 tc.tile_pool(name=f"ps2{h}", bufs=2, space="PSUM") as psp2:
                kx = bigp.tile([128, 2 * N], bf16, tag="kx")
                qx = bigp.tile([128, N], bf16, tag="qx")
                vx = bigp.tile([64, 257 * 132], bf16, tag="vx")
                nc.sync.dma_start(out=kx[:], in_=ins[f"kx_{h}"][:])
                nc.sync.dma_start(out=qx[:], in_=ins[f"qx_{h}"][:])
                vxv = vx[:].rearrange("p (w m) -> p w m", m=132)
                nc.sync.dma_start(
                    out=vxv, in_=ins[f"vx_{h}"][:].rearrange("(w p) m -> p w m", p=64))
                kp_t = bigp.tile([128, NCH], mybir.dt.float32, tag="kp_t")
                nc.sync.dma_start(out=kp_t[:], in_=ins[f"kp_{h}"][:])
                ones1 = bigp.tile([1, 128], mybir.dt.float32, tag="ones1")
                nc.vector.memset(ones1[:], 1.0)

                for blk in range(NBLK2):
                    dps = psp.tile([128, BLK * CHUNK], f32, tag="dps")
                    # MM1: per chunk j, keys = sorted slots [j*64, j*64+128) (ext),
                    # queries chunk j. lhsT = kx slice, rhs = qx chunk.
                    for jj in range(BLK):
                        j = blk * BLK + jj
                        nc.tensor.matmul(
                            dps[:, jj * CHUNK:(jj + 1) * CHUNK],
                            kx[:, j * 128:(j + 1) * 128],
                            qx[:, j * CHUNK:(j + 1) * CHUNK],
                            start=True, stop=True)
                    # exp
                    ex = wkp.tile([128, BLK * CHUNK], f32, tag="ex")
                    nc.scalar.activation(ex[:], dps[:],
                                         mybir.ActivationFunctionType.Exp)
                    # mask built on device: qpos (bcast via K=1 matmul) > kpos
                    qrow = wkp.tile([1, BLK * CHUNK], mybir.dt.float32, tag="qrow")
                    nc.sync.dma_start(
                        out=qrow[:],
                        in_=ins[f"stf_{h}"][:, blk * BLK * CHUNK:(blk + 1) * BLK * CHUNK])
                    qps = psp.tile([128, BLK * CHUNK], f32, tag="qps")
                    nc.tensor.matmul(qps[:], ones1[:], qrow[:], start=True, stop=True)
                    mks = wkp.tile([128, BLK * CHUNK], bf16, tag="mks")
                    for jj in range(BLK):
                        nc.vector.tensor_tensor(
                            out=mks[:, jj * CHUNK:(jj + 1) * CHUNK],
                            in0=qps[:, jj * CHUNK:(jj + 1) * CHUNK],
                            in1=kp_t[:, blk * BLK + jj:blk * BLK + jj + 1
                                     ].broadcast_to([128, CHUNK]),
                            op=mybir.AluOpType.is_gt)
                    at_lo = wkp.tile([64, BLK * CHUNK], bf16, tag="at_lo")
                    at_hi = wkp.tile([64, BLK * CHUNK], bf16, tag="at_hi")
                    nc.vector.tensor_tensor(out=at_lo[:], in0=ex[0:64, :], in1=mks[0:64, :],
                                            op=mybir.AluOpType.mult)
                    nc.vector.tensor_tensor(out=at_hi[:], in0=ex[64:128, :], in1=mks[64:128, :],
                                            op=mybir.AluOpType.mult)
                    # MM2 halves: out[q,d] += A_half.T @ V_half
                    for jj in range(0, BLK, 2):
                        j = blk * BLK + jj
                        ops = psp2.tile([128, 132], f32, tag="ops")
                        for q2 in range(2):  # two chunks -> two 64-row halves of psum
                            jq = j + q2
                            for half in range(2):  # key half w -> ext 64-row block jq+half
                                w = jq + half
                                at_sel = at_lo if (w % 2) == 0 else at_hi
                                a_h = at_sel[:, (jj + q2) * CHUNK:(jj + q2 + 1) * CHUNK]
                                v_h = vxv[:, w, :]
                                nc.tensor.matmul(ops[q2 * 64:q2 * 64 + 64, :],
                                                 a_h, v_h,
                                                 start=(half == 0), stop=(half == 1))
                        ob = wkp.tile([128, 132], bf16, tag="ob")
                        nc.vector.tensor_copy(ob[:], ops[:])
                        nc.sync.dma_start(
                            out=ins[f"out_{h}"][j * CHUNK:(j + 2) * CHUNK, :].rearrange(
                                "(a p) m -> p (a m)", p=128),
                            in_=ob[:])
    nc.finalize()
    return nc


_NC1 = None
_NC2 = None
EXEC_NS = []   # per-launch exec_time_ns when BASS_TRACE is set
TRACE_DIRS = []

# SRC_IDX[j, p] = ext slot index of the key row that chunk j places at
# within-chunk column p (the (j*64)%128 rotation of the baseline layout)
_J = np.arange(NCH)[:, None]
_P = np.arange(128)[None, :]
SRC_IDX = _J * CHUNK + ((_P - _J * CHUNK) % 128)


def _run(nc, in_maps, tag=""):
    import time
    t0 = time.time()
    r = run_bass_kernel_spmd(nc, in_maps, list(range(8)))
    EXEC_NS.append((tag, time.time() - t0))
    return r


def kernel(hidden_states, w_qk, w_v, rotations):
    global _NC1, _NC2
    hidden_states = np.asarray(hidden_states, dtype=np.float32)
    w_qk = np.asarray(w_qk, dtype=np.float32)
    w_v = np.asarray(w_v, dtype=np.float32)
    rotations = np.asarray(rotations, dtype=np.float32)

    # ---- launch 1: projections ----
    if _NC1 is None:
        _NC1 = build_nc1()
    in_maps1 = []
    for core in range(8):
        b = core // 4
        hp = core % 4  # head pair
        rows = slice(2 * hp * D, (2 * hp + 2) * D)
        wq = w_qk[rows, :].T.copy()          # [HID, 256]
        wqh, wql = _split_bf16(wq)
        in_maps1.append({
            "hT": np.ascontiguousarray(hidden_states[b].T),
            "wq_hi": wqh, "wq_lo": wql,
            "wv": np.ascontiguousarray(w_v[rows, :].T),
        })
    res1 = _run(_NC1, in_maps1, "nc1").results

    # ---- host middle: hashing, sort, layout prep (integer/index bookkeeping) ----
    rot2 = rotations.reshape(D, NH * (NB // 2))        # [128, 512]
    in_maps2 = []
    host_ctx = []
    for core in range(8):
        qk2 = res1[core]["qk_out"]          # [256, S] = qk^T two heads
        v2 = res1[core]["v_out"]
        m2 = {}
        ctx = []
        for h in range(2):
            qkT = qk2[h * D:(h + 1) * D, :]           # [128, S]
            qk = qkT.T                                 # [S, 128]
            v = v2[h * D:(h + 1) * D, :].T             # [S, 128]
            # hashing exactly like reference
            r = qk @ rot2                              # [S, 512]
            r = r.reshape(S, NH, NB // 2).transpose(1, 0, 2)   # [NH, S, 128]
            rc = np.concatenate([r, -r], axis=-1)      # [NH, S, 256]
            buckets = np.argmax(rc, axis=-1)           # [NH, S]
            buckets = buckets + (np.arange(NH) * NB)[:, None]
            flat = buckets.reshape(NH * S)
            sorted_idx = np.argsort(flat, kind="stable")       # [N]
            st = (sorted_idx % S).astype(np.int64)
            # normalized keys
            s_tok = (1.0 / np.sqrt(np.mean(qk * qk, axis=-1) + EPS)
                     / np.sqrt(np.float32(D))).astype(np.float32)
            k_norm = qk * s_tok[:, None]
            st_ext = np.concatenate([st[-CHUNK:], st])         # [64+N]
            # chunk j's window occupies kx cols [j*128, (j+1)*128); the row
            # landing at within-chunk col p is ext slot j*64 + ((p-j*64)%128).
            # SRC[j, p] is that ext slot -- one fancy-index builds kx/kpos.
            kbf = _bf(k_norm)[st_ext]                          # [64+N, 128] bf16
            kxm = kbf[SRC_IDX]                                 # [NCH, 128, 128]
            kx = np.ascontiguousarray(
                kxm.transpose(2, 0, 1).reshape(128, NCH * 128))
            qx = np.ascontiguousarray(_bf(qk)[st].T)           # [128, N]
            vs = v[st_ext]                                     # [64+N, 128]
            vx = np.zeros((257 * 64, 132), dtype=ml_dtypes.bfloat16)
            vx[:64 + N, :D] = _bf(vs)
            vx[:64 + N, D] = 1.0
            kp_host = np.ascontiguousarray(
                st_ext[SRC_IDX].transpose(1, 0).astype(np.float32))
            m2[f"kx_{h}"] = kx
            m2[f"qx_{h}"] = qx
            m2[f"vx_{h}"] = vx
            m2[f"kp_{h}"] = kp_host
            m2[f"stf_{h}"] = st.astype(np.float32)[None, :]
            ctx.append((st, v))
        in_maps2.append(m2)
        host_ctx.append(ctx)

    if _NC2 is None:
        _NC2 = build_nc2()
    res2 = _run(_NC2, in_maps2, "nc2").results

    # ---- host: unsort, combine hash rounds ----
    out = np.zeros((B, S, HID), dtype=np.float32)
    for core in range(8):
        b = core // 4
        hp = core % 4
        for h in range(2):
            st, v = host_ctx[core][h]
            o = res2[core][f"out_{h}"].astype(np.float32)      # [N, 132]
            ou = o[:, :D].reshape(NH, S, D)
            sm = o[:, D].reshape(NH, S)
            st4 = st.reshape(NH, S)
            # unsort each round
            ou_o = np.empty_like(ou)
            sm_o = np.empty_like(sm)
            for n in range(NH):
                ou_o[n, st4[n]] = ou[n]
                sm_o[n, st4[n]] = sm[n]
            lg = np.log(np.maximum(sm_o, 1e-38))
            lse = np.logaddexp.reduce(lg, axis=0)
            w = np.exp(lg - lse) / np.maximum(sm_o, 1e-38)     # [NH, S]
            res = np.sum(ou_o * w[:, :, None], axis=0)         # [S, D]
            dead = np.all(sm_o <= 1e-37, axis=0)
            if dead.any():
                res[dead] = v[dead]
            out[b, :, (2 * hp + h) * D:(2 * hp + h + 1) * D] = res
    return out

